# revision 1
# baseline (speedup 1.0000x reference)
"""Trainium2 8-core kernel for nn_AttentionLayer (GNN edge message passing).

Strategy (one SPMD graph, 8 NeuronCores):
  - Shard triplets by destination (idx_ji // 12500 -> owner core). Each core
    owns 12500 output nodes and all triplets writing to them, so no
    all-reduce of the aggregated messages is needed.
  - On-device: project q,k,v for the core's 12500 feats rows (3 matmuls/tile),
    AllGather the interleaved [q|v] table in 4 row-chunks (each chunk table
    <= 25600 rows so dma_gather's int16 indices reach every row).
  - Triplets sorted by (kj-chunk, ji). dma_gather fetches q|v rows (by kj,
    from the AG'd chunk tables) and k rows (by local ji). att=exp(lrelu(q.k))
    per 16-wide head on DVE+ACT.
  - Segment reduction WITHOUT scatter-add (dma_scatter_add races on duplicate
    indices): destinations are sorted, so each 128-token tile feeds one or two
    128-node windows. A bf16 selection matrix S[t,j] = (ji%256 == frame[j])
    turns the reduction into a TensorE matmul accumulated in PSUM; [att*v|att]
    is the moving operand so the denominator rides along for free.
  - Epilogue per window: normalize by denominator, transpose, 2-layer MLP
    (fused bias+relu), transpose back, add residual v, DMA the output shard.
  - Cell (chunk,window) sizes are padded to the max across all 8 cores so a
    single graph serves every core; pad tokens gather row 0 and carry
    ji-tag -1 which matches no selection frame.
"""

import numpy as np

import concourse.bass as bass
import concourse.tile as tile
from concourse import bacc, mybir
from concourse.bass_utils import run_bass_kernel_spmd

N = 100000
M = 800000
HID = 128
HEADS = 8
DH = 16
CORES = 8
NSH = N // CORES            # 12500 nodes per core
CH = (3200, 3200, 3200, 2900)   # local rows per AG chunk (sum = NSH)
CHOFF = (0, 3200, 6400, 9600)
NW = (NSH + 127) // 128     # 98 windows of 128 nodes
GRP = 1024                  # tokens per gather group
LOWP = True                 # bf16 q/v/k tables + att (halves AG + qv-gather bytes)
F32 = mybir.dt.float32
BF16 = mybir.dt.bfloat16
I16 = mybir.dt.int16

_CACHE = {}


def _host_prep(idx_kj, idx_ji):
    """Sort/shard/pad triplets. Returns shared structure + per-core arrays."""
    kj = np.asarray(idx_kj, dtype=np.int64)
    ji = np.asarray(idx_ji, dtype=np.int64)

    owner = ji // NSH
    # counts per (core, chunk, window)
    per_core = []
    counts = np.zeros((CORES, 4, NW), dtype=np.int64)
    for r in range(CORES):
        m = owner == r
        kj_r = kj[m]
        ji_l = ji[m] - r * NSH
        ow = kj_r // NSH
        j = kj_r % NSH
        c = np.minimum(j // 3200, 3)
        row16 = ow * np.array(CH, dtype=np.int64)[c] + (j - np.array(CHOFF)[c])
        w = ji_l // 128
        order = np.lexsort((ji_l, c))
        c, w, ji_l, row16 = c[order], w[order], ji_l[order], row16[order]
        np.add.at(counts[r], (c, w), 1)
        per_core.append((c, ji_l, row16))

    s_cw = np.maximum(counts.max(axis=0), 128)          # shared cell sizes
    # padded chunk run lengths (group aligned)
    L = [int(-(-int(s_cw[c].sum()) // GRP) * GRP) for c in range(4)]
    T = sum(L)

    # shared cell layout: global [start, end) per (c, w)
    cell_start = np.zeros((4, NW), dtype=np.int64)
    pos = 0
    chunk_start = []
    for c in range(4):
        chunk_start.append(pos)
        for w in range(NW):
            cell_start[c, w] = pos
            pos += int(s_cw[c, w])
        pos = chunk_start[c] + L[c]
    assert pos == T

    # per-core padded token arrays
    qv_idx = np.zeros((CORES, T), dtype=np.int16)
    k_idx = np.zeros((CORES, T), dtype=np.int16)
    jif = np.full((CORES, T), -1.0, dtype=np.float32)
    for r in range(CORES):
        c_r, ji_l, row16 = per_core[r]
        # tokens already sorted by (c, ji_l); write each cell's tokens
        base = 0
        for c in range(4):
            nc_tok = counts[r, c].sum()
            cc = slice(base, base + nc_tok)
            w_r = ji_l[cc] // 128
            # position within cell = running index among same (c,w)
            off_in_cell = np.arange(nc_tok) - np.concatenate(
                ([0], np.cumsum(counts[r, c])))[w_r]
            gpos = cell_start[c, w_r] + off_in_cell
            qv_idx[r, gpos] = row16[cc]
            k_idx[r, gpos] = ji_l[cc]
            jif[r, gpos] = (ji_l[cc] % 256).astype(np.float32)
            base += nc_tok

    def wrap16(a):  # token t -> [t % 16, t // 16], replicated to 128 parts
        return np.tile(np.ascontiguousarray(a.reshape(-1, 16).T), (8, 1))

    def tile128(a):  # token t -> [t % 128, t // 128]
        return np.ascontiguousarray(a.reshape(-1, 128).T)

    per_core_inputs = []
    for r in range(CORES):
        per_core_inputs.append({
            "qv_idx": wrap16(qv_idx[r]),
            "k_idx": wrap16(k_idx[r]),
            "jif": tile128(jif[r]).astype(np.float32),  # cast to bf16 via ml_dtypes later
        })
    struct = {
        "s_cw": s_cw, "L": tuple(L), "T": T,
        "cell_start": cell_start, "chunk_start": tuple(chunk_start),
    }
    return struct, per_core_inputs


def _build(struct):
    s_cw = struct["s_cw"]
    L = struct["L"]
    T = struct["T"]
    cell_start = struct["cell_start"]
    chunk_start = struct["chunk_start"]

    nc = bacc.Bacc(None, target_bir_lowering=False, debug=False)

    featsT = nc.dram_tensor("featsT", [128, NSH], F32, kind="ExternalInput")
    wqt = nc.dram_tensor("WqT", [128, 128], F32, kind="ExternalInput")
    wkt = nc.dram_tensor("WkT", [128, 128], F32, kind="ExternalInput")
    wvt = nc.dram_tensor("WvT", [128, 128], F32, kind="ExternalInput")
    w1t = nc.dram_tensor("W1T", [128, 128], F32, kind="ExternalInput")
    w2t = nc.dram_tensor("W2T", [128, 128], F32, kind="ExternalInput")
    b1d = nc.dram_tensor("b1", [128, 1], F32, kind="ExternalInput")
    b2d = nc.dram_tensor("b2", [128, 1], F32, kind="ExternalInput")
    identd = nc.dram_tensor("ident", [128, 128], F32, kind="ExternalInput")
    iotad = nc.dram_tensor("iota", [2, 128, (GRP // 128) * 128], BF16, kind="ExternalInput")
    qv_idx_d = nc.dram_tensor("qv_idx", [128, T // 16], I16, kind="ExternalInput")
    k_idx_d = nc.dram_tensor("k_idx", [128, T // 16], I16, kind="ExternalInput")
    jif_d = nc.dram_tensor("jif", [128, T // 128], F32, kind="ExternalInput")
    out_d = nc.dram_tensor("out", [NSH, 128], F32, kind="ExternalOutput")

    QDT = BF16 if LOWP else F32
    qv_b = [nc.dram_tensor(f"qvb{c}", [CH[c], 256], QDT) for c in range(4)]
    qv_t = [nc.dram_tensor(f"qvt{c}", [CORES * CH[c], 256], QDT,
                           addr_space="Shared") for c in range(4)]
    k_loc = nc.dram_tensor("k_loc", [NSH, 128], QDT)
    v_loc = nc.dram_tensor("v_loc", [NSH, 128], F32)

    with tile.TileContext(nc) as tc:
        with (
            tc.tile_pool(name="consts", bufs=1) as cp,
            tc.tile_pool(name="proj", bufs=3) as pp,
            tc.tile_pool(name="mainio", bufs=3) as mp,
            tc.tile_pool(name="work", bufs=2) as wp,
            tc.tile_pool(name="smat", bufs=4) as sp,
            tc.tile_pool(name="agg", bufs=1) as ap_,
            tc.tile_pool(name="epi", bufs=2) as ep,
            tc.tile_pool(name="psmm", bufs=5, space="PSUM") as ps_mm,
            tc.tile_pool(name="pswin", bufs=3, space="PSUM") as ps_w,
        ):
            # ---- constants into SBUF
            wq_sb = cp.tile([128, 128], F32, tag="wq")
            wk_sb = cp.tile([128, 128], F32, tag="wk")
            wv_sb = cp.tile([128, 128], F32, tag="wv")
            w1_sb = cp.tile([128, 128], F32, tag="w1")
            w2_sb = cp.tile([128, 128], F32, tag="w2")
            b1_sb = cp.tile([128, 1], F32, tag="b1")
            b2_sb = cp.tile([128, 1], F32, tag="b2")
            id_sb = cp.tile([128, 128], F32, tag="id")
            ioE_sb = cp.tile([128, GRP // 128, 128], BF16, tag="ioE")
            ioO_sb = cp.tile([128, GRP // 128, 128], BF16, tag="ioO")
            nc.sync.dma_start(out=wq_sb[:], in_=wqt[:, :])
            nc.sync.dma_start(out=wk_sb[:], in_=wkt[:, :])
            nc.sync.dma_start(out=wv_sb[:], in_=wvt[:, :])
            nc.sync.dma_start(out=w1_sb[:], in_=w1t[:, :])
            nc.sync.dma_start(out=w2_sb[:], in_=w2t[:, :])
            nc.sync.dma_start(out=b1_sb[:], in_=b1d[:, :])
            nc.sync.dma_start(out=b2_sb[:], in_=b2d[:, :])
            nc.sync.dma_start(out=id_sb[:], in_=identd[:, :])
            nc.sync.dma_start(out=ioE_sb[:].rearrange("p s j -> p (s j)"), in_=iotad[0, :, :])
            nc.sync.dma_start(out=ioO_sb[:].rearrange("p s j -> p (s j)"), in_=iotad[1, :, :])

            # ---- projections: per 128-row tile of this core's shard
            ntile = (NSH + 127) // 128
            for i in range(ntile):
                r0 = i * 128
                nr = min(128, NSH - r0)
                xT = pp.tile([128, 128], F32, tag="xT")
                nc.sync.dma_start(out=xT[:, :nr], in_=featsT[:, r0:r0 + nr])
                pq = ps_mm.tile([128, 128], F32, tag="mm")
                pk = ps_mm.tile([128, 128], F32, tag="mm")
                pv = ps_mm.tile([128, 128], F32, tag="mm")
                nc.tensor.matmul(pq[:nr, :], xT[:, :nr], wq_sb[:], start=True, stop=True)
                nc.tensor.matmul(pk[:nr, :], xT[:, :nr], wk_sb[:], start=True, stop=True)
                nc.tensor.matmul(pv[:nr, :], xT[:, :nr], wv_sb[:], start=True, stop=True)
                qv_sb = pp.tile([128, 256], QDT, tag="qv")
                k_sb = pp.tile([128, 128], QDT, tag="ksb")
                v_sb = pp.tile([128, 128], F32, tag="vsb32")
                nc.scalar.copy(out=qv_sb[:nr, 0:128], in_=pq[:nr, :])
                nc.scalar.copy(out=qv_sb[:nr, 128:256], in_=pv[:nr, :])
                nc.scalar.copy(out=k_sb[:nr, :], in_=pk[:nr, :])
                nc.vector.tensor_copy(out=v_sb[:nr, :], in_=pv[:nr, :])
                c = min(r0 // 3200, 3)
                nc.sync.dma_start(out=qv_b[c][r0 - CHOFF[c]:r0 - CHOFF[c] + nr, :],
                                  in_=qv_sb[:nr, :])
                nc.sync.dma_start(out=k_loc[r0:r0 + nr, :], in_=k_sb[:nr, :])
                nc.sync.dma_start(out=v_loc[r0:r0 + nr, :], in_=v_sb[:nr, :])

            # ---- all-gather the q|v table, chunk by chunk
            for c in range(4):
                nc.gpsimd.collective_compute(
                    "AllGather",
                    mybir.AluOpType.bypass,
                    ins=[qv_b[c].ap().opt()],
                    outs=[qv_t[c].ap().opt()],
                    replica_groups=[list(range(CORES))],
                )

            # ---- main loop
            # per chunk: cells (w, start, end); groups of GRP tokens
            agg_tiles = {}
            first_flush = {}
            for w in range(NW):
                agg_tiles[w] = ap_.tile([128, 136], F32, tag=f"agg{w}", name=f"agg{w}")
                first_flush[w] = True

            for c in range(4):
                cs = chunk_start[c]
                ngrp = L[c] // GRP
                cells = [(w, int(cell_start[c][w]),
                          int(cell_start[c][w] + s_cw[c][w])) for w in range(NW)]
                # map: tile index (global) -> list of (w, is_first, is_last)
                tile_mm = {}
                for (w, s, e) in cells:
                    t0, t1 = s // 128, (e - 1) // 128
                    for t in range(t0, t1 + 1):
                        tile_mm.setdefault(t, []).append(
                            (w, t == t0, t == t1))
                psum_w = {}
                for g in range(ngrp):
                    g0 = cs + g * GRP
                    col16 = g0 // 16
                    col128 = g0 // 128
                    iq = mp.tile([128, GRP // 16], I16, tag="iq")
                    ik = mp.tile([128, GRP // 16], I16, tag="ik")
                    jf = mp.tile([128, GRP // 128], F32, tag="jf")
                    nc.sync.dma_start(out=iq[:], in_=qv_idx_d[:, col16:col16 + GRP // 16])
                    nc.sync.dma_start(out=ik[:], in_=k_idx_d[:, col16:col16 + GRP // 16])
                    nc.sync.dma_start(out=jf[:], in_=jif_d[:, col128:col128 + GRP // 128])

                    qv_g = mp.tile([128, GRP // 128, 256], QDT, tag="qvg")
                    k_g = mp.tile([128, GRP // 128, 128], QDT, tag="kg")
                    nc.gpsimd.dma_gather(
                        out_ap=qv_g[:], in_ap=qv_t[c][:, :], idxs_ap=iq[:],
                        num_idxs=GRP, num_idxs_reg=GRP, elem_size=256)
                    nc.gpsimd.dma_gather(
                        out_ap=k_g[:], in_ap=k_loc[:, :], idxs_ap=ik[:],
                        num_idxs=GRP, num_idxs_reg=GRP, elem_size=128)

                    ns = GRP // 128  # slots per partition
                    qk = wp.tile([128, ns, 128], QDT, tag="qk")
                    nc.vector.tensor_tensor(
                        out=qk[:], in0=qv_g[:, :, 0:128], in1=k_g[:],
                        op=mybir.AluOpType.mult)
                    att = wp.tile([128, ns, HEADS], F32, tag="att")
                    nc.vector.tensor_reduce(
                        out=att[:],
                        in_=qk[:].rearrange("p s (h d) -> p s h d", d=DH),
                        axis=mybir.AxisListType.X, op=mybir.AluOpType.add)
                    attl = wp.tile([128, ns, HEADS], F32, tag="attl")
                    nc.scalar.activation(
                        out=attl[:], in_=att[:],
                        func=mybir.ActivationFunctionType.Lrelu, alpha=0.2)
                    atte = wp.tile([128, ns, HEADS], QDT, tag="atte")
                    nc.scalar.activation(
                        out=atte[:], in_=attl[:],
                        func=mybir.ActivationFunctionType.Exp)
                    rhs = wp.tile([128, ns, 136], BF16, tag="rhs")
                    nc.vector.tensor_tensor(
                        out=rhs[:, :, 0:128].rearrange("p s (h d) -> p s h d", d=DH),
                        in0=qv_g[:, :, 128:256].rearrange("p s (h d) -> p s h d", d=DH),
                        in1=atte[:].to_broadcast([128, ns, HEADS, DH]),
                        op=mybir.AluOpType.mult)
                    nc.vector.tensor_copy(out=rhs[:, :, 128:136], in_=atte[:])
                    S_par = {}
                    need_par = set()
                    for t_loc in range(ns):
                        for (w, _f, _l) in tile_mm.get((g0 // 128) + t_loc, []):
                            need_par.add(w % 2)
                    for par in sorted(need_par):
                        St = sp.tile([128, ns, 128], BF16, tag=f"S{par}",
                                     name=f"S{par}_c{c}_g{g}")
                        io = ioE_sb if par == 0 else ioO_sb
                        nc.vector.tensor_tensor(
                            out=St[:], in0=io[:],
                            in1=jf[:].rearrange("p (s o) -> p s o", o=1).to_broadcast(
                                [128, ns, 128]),
                            op=mybir.AluOpType.is_equal)
                        S_par[par] = St

                    # segment matmuls for tiles in this group
                    for t_loc in range(ns):
                        t_glob = (g0 // 128) + t_loc
                        for (w, first, last) in tile_mm.get(t_glob, []):
                            if w not in psum_w:
                                psum_w[w] = ps_w.tile([128, 136], F32, tag="pw", name=f"pw_c{c}_w{w}")
                            nc.tensor.matmul(
                                psum_w[w][:], S_par[w % 2][:, t_loc, :],
                                rhs[:, t_loc, :],
                                start=first, stop=last, skip_group_check=True)
                            if last:
                                if first_flush[w]:
                                    nc.vector.tensor_copy(
                                        out=agg_tiles[w][:], in_=psum_w[w][:])
                                    first_flush[w] = False
                                else:
                                    nc.vector.tensor_tensor(
                                        out=agg_tiles[w][:], in0=agg_tiles[w][:],
                                        in1=psum_w[w][:], op=mybir.AluOpType.add)
                                del psum_w[w]

            # ---- epilogue per window
            for w in range(NW):
                nw_ = min(128, NSH - w * 128)
                acc = agg_tiles[w]
                den = ep.tile([128, HEADS], F32, tag="den")
                nc.vector.tensor_scalar(
                    out=den[:], in0=acc[:, 128:136], scalar1=1e-30, scalar2=None,
                    op0=mybir.AluOpType.max)
                rec = ep.tile([128, HEADS], F32, tag="rec")
                nc.vector.reciprocal(out=rec[:], in_=den[:])
                aggn = ep.tile([128, 128], F32, tag="aggn")
                nc.vector.tensor_tensor(
                    out=aggn[:].rearrange("p (h d) -> p h d", d=DH),
                    in0=acc[:, 0:128].rearrange("p (h d) -> p h d", d=DH),
                    in1=rec[:].to_broadcast([128, HEADS, DH]),
                    op=mybir.AluOpType.mult)
                paT = ps_mm.tile([128, 128], F32, tag="mm")
                nc.tensor.transpose(out=paT[:], in_=aggn[:], identity=id_sb[:])
                aT = ep.tile([128, 128], F32, tag="aT")
                nc.vector.tensor_copy(out=aT[:], in_=paT[:])
                ph1 = ps_mm.tile([128, 128], F32, tag="mm")
                nc.tensor.matmul(ph1[:], w1_sb[:], aT[:], start=True, stop=True)
                h1 = ep.tile([128, 128], F32, tag="h1")
                nc.scalar.activation(
                    out=h1[:], in_=ph1[:],
                    func=mybir.ActivationFunctionType.Relu, bias=b1_sb[:, 0:1])
                ph2 = ps_mm.tile([128, 128], F32, tag="mm")
                nc.tensor.matmul(ph2[:], w2_sb[:], h1[:], start=True, stop=True)
                h2 = ep.tile([128, 128], F32, tag="h2")
                nc.scalar.activation(
                    out=h2[:], in_=ph2[:],
                    func=mybir.ActivationFunctionType.Relu, bias=b2_sb[:, 0:1])
                pho = ps_mm.tile([128, 128], F32, tag="mm")
                nc.tensor.transpose(out=pho[:], in_=h2[:], identity=id_sb[:])
                vsb = ep.tile([128, 128], F32, tag="vsb")
                nc.sync.dma_start(out=vsb[:nw_, :], in_=v_loc[w * 128:w * 128 + nw_, :])
                osb = ep.tile([128, 128], F32, tag="osb")
                nc.vector.tensor_tensor(
                    out=osb[:nw_, :], in0=pho[:nw_, :], in1=vsb[:nw_, :],
                    op=mybir.AluOpType.add)
                nc.sync.dma_start(out=out_d[w * 128:w * 128 + nw_, :], in_=osb[:nw_, :])

    nc.compile()
    return nc


def kernel(feats, idx_kj, idx_ji, Wv, Wq, Wk, W1, b1, W2, b2):
    import ml_dtypes

    feats = np.asarray(feats, dtype=np.float32)
    struct, per_core = _host_prep(idx_kj, idx_ji)

    key = (struct["T"],) + struct["L"] + tuple(struct["s_cw"].ravel())
    if key in _CACHE:
        nc = _CACHE[key]
    else:
        nc = _build(struct)
        _CACHE[key] = nc

    ns_ = GRP // 128
    iota = np.zeros((2, 128, ns_ * 128), dtype=ml_dtypes.bfloat16)
    iota[0] = np.broadcast_to(np.tile(np.arange(128, dtype=np.float32), ns_), (128, ns_ * 128))
    iota[1] = np.broadcast_to(np.tile(np.arange(128, 256, dtype=np.float32), ns_), (128, ns_ * 128))

    common = {
        "WqT": np.ascontiguousarray(np.asarray(Wq, np.float32).T),
        "WkT": np.ascontiguousarray(np.asarray(Wk, np.float32).T),
        "WvT": np.ascontiguousarray(np.asarray(Wv, np.float32).T),
        "W1T": np.ascontiguousarray(np.asarray(W1, np.float32).T),
        "W2T": np.ascontiguousarray(np.asarray(W2, np.float32).T),
        "b1": np.asarray(b1, np.float32).reshape(128, 1),
        "b2": np.asarray(b2, np.float32).reshape(128, 1),
        "ident": np.eye(128, dtype=np.float32),
        "iota": iota,
    }
    in_maps = []
    for r in range(CORES):
        m = dict(common)
        m["featsT"] = np.ascontiguousarray(feats[r * NSH:(r + 1) * NSH].T)
        m["qv_idx"] = per_core[r]["qv_idx"]
        m["k_idx"] = per_core[r]["k_idx"]
        m["jif"] = per_core[r]["jif"]
        in_maps.append(m)

    res = run_bass_kernel_spmd(nc, in_maps, core_ids=list(range(CORES)))
    global _LAST_RESULTS
    _LAST_RESULTS = res
    out = np.concatenate([np.asarray(res.results[r]["out"]) for r in range(CORES)], axis=0)
    return out.astype(np.float32)


_LAST_RESULTS = None



# revision 11
# speedup vs baseline: 1.7674x; 1.7674x over previous
"""Trainium2 8-core kernel for nn_AttentionLayer (GNN edge message passing).

Strategy (one SPMD graph, 8 NeuronCores):
  - Shard triplets by destination (idx_ji // 12500 -> owner core). Each core
    owns 12500 output nodes and all triplets writing to them.
  - On-device: project q,k,v for the core's 12500 feats rows; AllGather the
    interleaved [q|v] bf16 table in 4 row-chunks (chunk tables <= 25600 rows
    so dma_gather's int16 indices reach every row), pipelined behind the
    projection tiles. k and v never leave the core: k lives in SBUF as a
    [node, hid] matmul-operand table, v in SBUF for the epilogue residual.
  - Main loop: 25 window-blocks (4 dst windows of 128 nodes each) x 4 source
    chunks. One dma_gather per (block, chunk) run fetches the per-edge [q|v]
    rows - the only SWDGE/Q7 work (~8.4ns/edge descriptor generation paces
    the kernel; the old second gather for k is gone).
  - Per-edge k rows come from SBUF via TensorE: K_sel = S_T.T @ K_w with
    host-shipped one-hot S_T (node x token) stationaries.
  - att = exp(lrelu(q.k)): lrelu on DVE as max(x, 0.2x), exp on ACT - the
    ACT table is loaded once (no Lrelu/Exp table thrash).
  - Segment reduction via TensorE selection matmuls (S[t,j] = (ji%256 ==
    frame[j]) built on DVE) accumulated entirely in PSUM across the block's
    4 chunk-cells; the denominator rides in columns 128:136.
  - Epilogue (normalize, transpose, f32 2-layer MLP with fused bias+relu on
    DVE, transpose back, +v residual) runs per window as soon as its PSUM
    closes, so it interleaves with the gathers instead of forming a tail.
  - Cell sizes are padded to the max across all 8 cores so a single graph
    serves every core; pad tokens gather row 0 and carry tag -1 which
    matches no selection frame.
"""

import numpy as np

import concourse.bass as bass
import concourse.tile as tile
from concourse import bacc, mybir
from concourse.bass_utils import run_bass_kernel_spmd

N = 100000
M = 800000
HID = 128
HEADS = 8
DH = 16
CORES = 8
NSH = N // CORES            # 12500 nodes per core
CH = (3200, 3200, 3200, 2900)   # local rows per AG chunk (sum = NSH)
CHOFF = (0, 3200, 6400, 9600)
NW = (NSH + 127) // 128     # 98 windows of 128 nodes
WB = 4                      # windows per block
NB = (NW + WB - 1) // WB    # 25 blocks
F32 = mybir.dt.float32
BF16 = mybir.dt.bfloat16
I16 = mybir.dt.int16

_CACHE = {}


def _structure(s_cw):
    """Static token/matrix layout shared by host prep and graph build.

    Token order: (block, chunk, window, pos-in-cell); every (block, chunk)
    run is padded to a 128 multiple. Each 128-token tile intersects <= 2
    cells (cells are >= 128 tokens), always of different window parity.
    """
    runs = []
    cellpos = {}
    T = 0
    nst = 0
    for B in range(NB):
        ws = list(range(B * WB, min((B + 1) * WB, NW)))
        for c in range(4):
            g0 = T
            st0 = nst
            # cell starts within the run
            starts = []
            pos = 0
            for w in ws:
                starts.append(pos)
                cellpos[(B, c, w)] = g0 + pos
                pos += int(s_cw[c][w])
            L = -(-pos // 128) * 128
            ns = L // 128
            tiles = []
            for t in range(ns):
                t0, t1 = t * 128, (t + 1) * 128
                contribs = []
                for wi, w in enumerate(ws):
                    cs = starts[wi]
                    ce = cs + int(s_cw[c][w])
                    if cs < t1 and ce > t0:
                        first_tile = cs >= t0      # cell starts in this tile
                        last_tile = ce <= t1       # cell ends in this tile
                        contribs.append([w, first_tile, last_tile])
                assert 1 <= len(contribs) <= 2
                if len(contribs) == 2:
                    assert contribs[0][0] % 2 != contribs[1][0] % 2
                tiles.append(contribs)
                nst += len(contribs)
            runs.append({
                "B": B, "c": c, "g0": g0, "L": L, "ns": ns,
                "st0": st0, "nst": nst - st0, "tiles": tiles,
            })
            T += L
    # psum first/last flags: window w's psum opens at its first contribution
    # in chunk 0 and closes at its last contribution in chunk 3.
    for run in runs:
        c = run["c"]
        for contribs in run["tiles"]:
            for e in contribs:
                w, first_tile, last_tile = e
                pf = (c == 0) and first_tile
                pl = (c == 3) and last_tile
                e[1] = pf
                e[2] = pl
    nsmax = max(r["ns"] for r in runs)
    nstrmax = max(r["nst"] for r in runs)
    # (tile, parity) -> S_T matrix index
    m_of = np.full((T // 128, 2), -1, dtype=np.int64)
    for run in runs:
        m = run["st0"]
        for t, contribs in enumerate(run["tiles"]):
            g = run["g0"] // 128 + t
            for (w, _pf, _pl) in contribs:
                m_of[g, w % 2] = m
                m += 1
    return {
        "s_cw": s_cw, "runs": runs, "cellpos": cellpos, "T": T,
        "nst": nst, "nsmax": nsmax, "nstrmax": nstrmax, "m_of": m_of,
    }


def _host_prep(idx_kj, idx_ji):
    import ml_dtypes

    kj = np.asarray(idx_kj, dtype=np.int64)
    ji = np.asarray(idx_ji, dtype=np.int64)
    owner = ji // NSH

    per_core = []
    counts = np.zeros((CORES, 4, NW), dtype=np.int64)
    chv = np.array(CH, dtype=np.int64)
    chof = np.array(CHOFF, dtype=np.int64)
    for r in range(CORES):
        m = owner == r
        kj_r = kj[m]
        ji_l = ji[m] - r * NSH
        ow = kj_r // NSH
        j = kj_r % NSH
        c = np.minimum(j // 3200, 3)
        row16 = ow * chv[c] + (j - chof[c])
        w = ji_l // 128
        np.add.at(counts[r], (c, w), 1)
        per_core.append((c, w, ji_l, row16))

    s_cw = np.maximum(counts.max(axis=0), 128)
    struct = _structure(s_cw)
    T = struct["T"]
    m_of = struct["m_of"]
    cellpos = struct["cellpos"]
    nst = struct["nst"]

    # (c, w) cell -> global token start, as a flat lookup
    cell_start = np.zeros((4, NW), dtype=np.int64)
    for c in range(4):
        for w in range(NW):
            cell_start[c, w] = cellpos[(w // WB, c, w)]

    def wrap16(a):
        return np.tile(np.ascontiguousarray(a.reshape(-1, 16).T), (8, 1))

    def tile128(a):
        return np.ascontiguousarray(a.reshape(-1, 128).T)

    per_core_inputs = []
    for r in range(CORES):
        c, w, ji_l, row16 = per_core[r]
        order = np.lexsort((ji_l, w, c, w // WB))
        c, w, ji_l, row16 = c[order], w[order], ji_l[order], row16[order]
        key = c * NW + w  # increasing within (B,c,w) sort? B=w//WB ordering:
        # sorted by (B, c, w): key (B, c, w) tuple-monotone; encode directly
        key = (w // WB) * 4 * NW + c * NW + w
        uniq, idx_start, cnt = np.unique(key, return_index=True,
                                         return_counts=True)
        off = np.arange(key.shape[0]) - np.repeat(idx_start, cnt)
        ub = uniq // (4 * NW)
        uc = (uniq // NW) % 4
        uw = uniq % NW
        ustart = cell_start[uc, uw]
        assert np.all(ub == uw // WB)
        gpos = np.repeat(ustart, cnt) + off

        qv_idx = np.zeros(T, dtype=np.int16)
        jif = np.full(T, -1.0, dtype=np.float32)
        qv_idx[gpos] = row16.astype(np.int16)
        jif[gpos] = (ji_l % 256).astype(np.float32)

        g = gpos // 128
        tloc = gpos % 128
        me = m_of[g, w % 2]
        assert np.all(me >= 0)
        st = np.zeros((nst, 128, 128), dtype=ml_dtypes.bfloat16)
        st[me, ji_l % 128, tloc] = 1.0
        st = np.ascontiguousarray(
            st.transpose(1, 0, 2).reshape(128, nst * 128))

        per_core_inputs.append({
            "qv_idx": wrap16(qv_idx),
            "k_idx_unused": None,
            "jif": tile128(jif),
            "st": st,
        })
    return struct, per_core_inputs


def _build(struct):
    runs = struct["runs"]
    T = struct["T"]
    NST = struct["nst"]
    NSMAX = struct["nsmax"]
    NSTRMAX = struct["nstrmax"]

    nc = bacc.Bacc(None, target_bir_lowering=False, debug=False)

    featsT = nc.dram_tensor("featsT", [128, NSH], F32, kind="ExternalInput")
    wqt = nc.dram_tensor("WqT", [128, 128], F32, kind="ExternalInput")
    wkt = nc.dram_tensor("WkT", [128, 128], F32, kind="ExternalInput")
    wvt = nc.dram_tensor("WvT", [128, 128], F32, kind="ExternalInput")
    w1t = nc.dram_tensor("W1T", [128, 128], F32, kind="ExternalInput")
    w2t = nc.dram_tensor("W2T", [128, 128], F32, kind="ExternalInput")
    b1d = nc.dram_tensor("b1", [128, 1], F32, kind="ExternalInput")
    b2d = nc.dram_tensor("b2", [128, 1], F32, kind="ExternalInput")
    identd = nc.dram_tensor("ident", [128, 128], F32, kind="ExternalInput")
    iotad = nc.dram_tensor("iota", [2, 128, NSMAX * 128], BF16,
                           kind="ExternalInput")
    qv_idx_d = nc.dram_tensor("qv_idx", [128, T // 16], I16,
                              kind="ExternalInput")
    jif_d = nc.dram_tensor("jif", [128, T // 128], F32, kind="ExternalInput")
    st_d = nc.dram_tensor("st", [128, NST * 128], BF16, kind="ExternalInput")
    out_d = nc.dram_tensor("out", [NSH, 128], F32, kind="ExternalOutput")

    qv_b = [nc.dram_tensor(f"qvb{c}", [CH[c], 256], BF16) for c in range(4)]
    qv_t = [nc.dram_tensor(f"qvt{c}", [CORES * CH[c], 256], BF16,
                           addr_space="Shared") for c in range(4)]

    with tile.TileContext(nc) as tc:
        with (
            tc.tile_pool(name="consts", bufs=1) as cp,
            tc.tile_pool(name="proj", bufs=3) as pp,
            tc.tile_pool(name="mainio", bufs=3) as mp,
            tc.tile_pool(name="work", bufs=2) as wp,
            tc.tile_pool(name="epi", bufs=2) as ep,
            tc.tile_pool(name="psw", bufs=4, space="PSUM") as ps_w,
            tc.tile_pool(name="psk", bufs=2, space="PSUM") as ps_k,
            tc.tile_pool(name="psm", bufs=2, space="PSUM") as ps_m,
        ):
            # ---- constants
            wq_sb = cp.tile([128, 128], F32, tag="wq")
            wk_sb = cp.tile([128, 128], F32, tag="wk")
            wv_sb = cp.tile([128, 128], F32, tag="wv")
            w1_sb = cp.tile([128, 128], F32, tag="w1")
            w2_sb = cp.tile([128, 128], F32, tag="w2")
            b1_sb = cp.tile([128, 1], F32, tag="b1")
            b2_sb = cp.tile([128, 1], F32, tag="b2")
            id_sb = cp.tile([128, 128], F32, tag="id")
            ioE_sb = cp.tile([128, NSMAX, 128], BF16, tag="ioE")
            ioO_sb = cp.tile([128, NSMAX, 128], BF16, tag="ioO")
            ktab = cp.tile([128, NW, 128], BF16, tag="ktab")
            vtab = cp.tile([128, NW, 128], F32, tag="vtab")
            nc.sync.dma_start(out=wq_sb[:], in_=wqt[:, :])
            nc.sync.dma_start(out=wk_sb[:], in_=wkt[:, :])
            nc.sync.dma_start(out=wv_sb[:], in_=wvt[:, :])
            nc.sync.dma_start(out=w1_sb[:], in_=w1t[:, :])
            nc.sync.dma_start(out=w2_sb[:], in_=w2t[:, :])
            nc.sync.dma_start(out=b1_sb[:], in_=b1d[:, :])
            nc.sync.dma_start(out=b2_sb[:], in_=b2d[:, :])
            nc.sync.dma_start(out=id_sb[:], in_=identd[:, :])
            nc.sync.dma_start(out=ioE_sb[:].rearrange("p s j -> p (s j)"),
                              in_=iotad[0, :, :])
            nc.sync.dma_start(out=ioO_sb[:].rearrange("p s j -> p (s j)"),
                              in_=iotad[1, :, :])

            # window 97 is partial (84 rows): zero ktab so K_sel matmuls
            # never read uninitialized node rows
            nc.vector.memset(ktab[:], 0)

            # ---- projections; AG chunk c as soon as its rows are done
            ntile = (NSH + 127) // 128
            ag_after = {24: 0, 49: 1, 74: 2, ntile - 1: 3}
            for i in range(ntile):
                r0 = i * 128
                nr = min(128, NSH - r0)
                xT = pp.tile([128, 128], F32, tag="xT")
                nc.sync.dma_start(out=xT[:, :nr], in_=featsT[:, r0:r0 + nr])
                pq = ps_m.tile([128, 128], F32, tag="mm")
                pk = ps_m.tile([128, 128], F32, tag="mm")
                pv = ps_k.tile([128, 128], F32, tag="ksel")
                nc.tensor.matmul(pq[:nr, :], xT[:, :nr], wq_sb[:],
                                 start=True, stop=True)
                nc.tensor.matmul(pk[:nr, :], xT[:, :nr], wk_sb[:],
                                 start=True, stop=True)
                nc.tensor.matmul(pv[:nr, :], xT[:, :nr], wv_sb[:],
                                 start=True, stop=True)
                qv_sb = pp.tile([128, 256], BF16, tag="qv")
                nc.scalar.copy(out=qv_sb[:nr, 0:128], in_=pq[:nr, :])
                nc.scalar.copy(out=qv_sb[:nr, 128:256], in_=pv[:nr, :])
                nc.scalar.copy(out=ktab[:nr, i, :], in_=pk[:nr, :])
                nc.vector.tensor_copy(out=vtab[:nr, i, :], in_=pv[:nr, :])
                c = min(r0 // 3200, 3)
                nc.sync.dma_start(
                    out=qv_b[c][r0 - CHOFF[c]:r0 - CHOFF[c] + nr, :],
                    in_=qv_sb[:nr, :])
            for cc in range(4):
                nc.gpsimd.collective_compute(
                    "AllGather",
                    mybir.AluOpType.bypass,
                    ins=[qv_b[cc].ap().opt()],
                    outs=[qv_t[cc].ap().opt()],
                    replica_groups=[list(range(CORES))],
                )

            # ---- main loop
            psum_w = {}

            def epilogue(w, pw):
                nw_ = min(128, NSH - w * 128)
                den = ep.tile([128, HEADS], F32, tag="den")
                nc.vector.tensor_scalar(
                    out=den[:], in0=pw[:, 128:136], scalar1=1e-30,
                    scalar2=None, op0=mybir.AluOpType.max)
                rec = ep.tile([128, HEADS], F32, tag="rec")
                nc.vector.reciprocal(out=rec[:], in_=den[:])
                aggn = ep.tile([128, 128], F32, tag="aggn")
                nc.vector.tensor_tensor(
                    out=aggn[:].rearrange("p (h d) -> p h d", d=DH),
                    in0=pw[:, 0:128].rearrange("p (h d) -> p h d", d=DH),
                    in1=rec[:].to_broadcast([128, HEADS, DH]),
                    op=mybir.AluOpType.mult)
                paT = ps_m.tile([128, 128], F32, tag="mm")
                nc.tensor.transpose(out=paT[:], in_=aggn[:], identity=id_sb[:])
                aT = ep.tile([128, 128], F32, tag="aT")
                nc.scalar.copy(out=aT[:], in_=paT[:])
                ph1 = ps_m.tile([128, 128], F32, tag="mm")
                nc.tensor.matmul(ph1[:], w1_sb[:], aT[:], start=True,
                                 stop=True)
                h1 = ep.tile([128, 128], F32, tag="h1")
                nc.vector.tensor_scalar(
                    out=h1[:], in0=ph1[:], scalar1=b1_sb[:, 0:1],
                    scalar2=0.0, op0=mybir.AluOpType.add,
                    op1=mybir.AluOpType.max)
                ph2 = ps_m.tile([128, 128], F32, tag="mm")
                nc.tensor.matmul(ph2[:], w2_sb[:], h1[:], start=True,
                                 stop=True)
                h2 = ep.tile([128, 128], F32, tag="h2")
                nc.vector.tensor_scalar(
                    out=h2[:], in0=ph2[:], scalar1=b2_sb[:, 0:1],
                    scalar2=0.0, op0=mybir.AluOpType.add,
                    op1=mybir.AluOpType.max)
                pho = ps_m.tile([128, 128], F32, tag="mm")
                nc.tensor.transpose(out=pho[:], in_=h2[:], identity=id_sb[:])
                osb = ep.tile([128, 128], F32, tag="osb")
                nc.vector.tensor_tensor(
                    out=osb[:nw_, :], in0=pho[:nw_, :],
                    in1=vtab[:nw_, w, :], op=mybir.AluOpType.add)
                nc.sync.dma_start(out=out_d[w * 128:w * 128 + nw_, :],
                                  in_=osb[:nw_, :])

            import os
            trunc = int(os.environ.get("KERNEL_TRUNC_BLOCKS", "0"))
            no_epi = os.environ.get("KERNEL_NO_EPI") == "1"
            no_ksel = os.environ.get("KERNEL_NO_KSEL") == "1"
            closed = []
            for run in runs:
                if trunc and run["B"] >= trunc:
                    continue
                c = run["c"]
                g0 = run["g0"]
                ns = run["ns"]
                st0 = run["st0"]
                nst_r = run["nst"]
                iq = mp.tile([128, NSMAX * 8], I16, tag="iq")
                jf = mp.tile([128, NSMAX], F32, tag="jf")
                stb = mp.tile([128, NSTRMAX * 128], BF16, tag="stb")
                nc.sync.dma_start(
                    out=iq[:, :ns * 8],
                    in_=qv_idx_d[:, g0 // 16:g0 // 16 + ns * 8])
                nc.sync.dma_start(
                    out=jf[:, :ns],
                    in_=jif_d[:, g0 // 128:g0 // 128 + ns])
                if not no_ksel:
                    nc.sync.dma_start(
                        out=stb[:, :nst_r * 128],
                        in_=st_d[:, st0 * 128:(st0 + nst_r) * 128])
                qv_g = mp.tile([128, NSMAX, 256], BF16, tag="qvg")
                # HW hangs on dma_gather with >1024 indices: chunk to <=8
                # slots per call (per-descriptor Q7 cost is unchanged).
                for s0 in range(0, ns, 8):
                    s1 = min(s0 + 8, ns)
                    nc.gpsimd.dma_gather(
                        out_ap=qv_g[:, s0:s1, :], in_ap=qv_t[c][:, :],
                        idxs_ap=iq[:, s0 * 8:s1 * 8],
                        num_idxs=(s1 - s0) * 128,
                        num_idxs_reg=(s1 - s0) * 128, elem_size=256)

                S_e = wp.tile([128, NSMAX, 128], BF16, tag="Se")
                S_o = wp.tile([128, NSMAX, 128], BF16, tag="So")
                jfb = jf[:, :ns].rearrange(
                    "p (s o) -> p s o", o=1).to_broadcast([128, ns, 128])
                nc.vector.tensor_tensor(
                    out=S_e[:, :ns, :], in0=ioE_sb[:, :ns, :], in1=jfb,
                    op=mybir.AluOpType.is_equal)
                nc.vector.tensor_tensor(
                    out=S_o[:, :ns, :], in0=ioO_sb[:, :ns, :], in1=jfb,
                    op=mybir.AluOpType.is_equal)

                # K_sel per tile via TensorE + qk product on DVE
                qk = wp.tile([128, NSMAX, 128], BF16, tag="qk")
                if no_ksel:
                    nc.vector.tensor_copy(out=qk[:, :ns, :],
                                          in_=qv_g[:, :ns, 0:128])
                else:
                    ki = st0
                    for t, contribs in enumerate(run["tiles"]):
                        ksel = ps_k.tile([128, 128], F32, tag="ksel")
                        nct = len(contribs)
                        for i, (w, _pf, _pl) in enumerate(contribs):
                            nc.tensor.matmul(
                                ksel[:],
                                stb[:, (ki - st0 + i) * 128:
                                    (ki - st0 + i + 1) * 128],
                                ktab[:, w, :],
                                start=(i == 0), stop=(i == nct - 1),
                                skip_group_check=True)
                        ki += nct
                        nc.vector.tensor_tensor(
                            out=qk[:, t, :], in0=ksel[:],
                            in1=qv_g[:, t, 0:128], op=mybir.AluOpType.mult)

                att = wp.tile([128, NSMAX, HEADS], F32, tag="att")
                nc.vector.tensor_reduce(
                    out=att[:, :ns, :],
                    in_=qk[:, :ns, :].rearrange("p s (h d) -> p s h d", d=DH),
                    axis=mybir.AxisListType.X, op=mybir.AluOpType.add)
                tmp = wp.tile([128, NSMAX, HEADS], F32, tag="tmp")
                nc.vector.tensor_scalar(
                    out=tmp[:, :ns, :], in0=att[:, :ns, :], scalar1=0.2,
                    scalar2=None, op0=mybir.AluOpType.mult)
                attl = wp.tile([128, NSMAX, HEADS], F32, tag="attl")
                nc.vector.tensor_tensor(
                    out=attl[:, :ns, :], in0=att[:, :ns, :],
                    in1=tmp[:, :ns, :], op=mybir.AluOpType.max)
                atte = wp.tile([128, NSMAX, HEADS], BF16, tag="atte")
                nc.scalar.activation(
                    out=atte[:, :ns, :], in_=attl[:, :ns, :],
                    func=mybir.ActivationFunctionType.Exp)
                rhs = wp.tile([128, NSMAX, 136], BF16, tag="rhs")
                nc.vector.tensor_tensor(
                    out=rhs[:, :ns, 0:128].rearrange(
                        "p s (h d) -> p s h d", d=DH),
                    in0=qv_g[:, :ns, 128:256].rearrange(
                        "p s (h d) -> p s h d", d=DH),
                    in1=atte[:, :ns, :].to_broadcast([128, ns, HEADS, DH]),
                    op=mybir.AluOpType.mult)
                nc.vector.tensor_copy(out=rhs[:, :ns, 128:136],
                                      in_=atte[:, :ns, :])

                for t, contribs in enumerate(run["tiles"]):
                    for (w, pf, pl) in contribs:
                        if pf:
                            assert w not in psum_w
                            psum_w[w] = ps_w.tile([128, 136], F32, tag="pw",
                                                  name=f"pw_w{w}")
                        S_x = S_e if w % 2 == 0 else S_o
                        nc.tensor.matmul(
                            psum_w[w][:], S_x[:, t, :], rhs[:, t, :],
                            start=pf, stop=pl, skip_group_check=True)
                        if pl:
                            closed.append((w, psum_w.pop(w)))

                # end of block: all 4 windows' PSUM groups are closed;
                # epilogues here never interleave transposes/MLP matmuls
                # with open accumulation groups.
                if c == 3:
                    for (w, pw) in closed:
                        if not no_epi:
                            epilogue(w, pw)
                    closed = []

            assert not psum_w

    nc.compile()
    return nc


def kernel(feats, idx_kj, idx_ji, Wv, Wq, Wk, W1, b1, W2, b2):
    import ml_dtypes

    feats = np.asarray(feats, dtype=np.float32)
    struct, per_core = _host_prep(idx_kj, idx_ji)

    key = (struct["T"], struct["nst"]) + tuple(struct["s_cw"].ravel())
    if key in _CACHE:
        nc = _CACHE[key]
    else:
        nc = _build(struct)
        _CACHE[key] = nc

    nsmax = struct["nsmax"]
    iota = np.zeros((2, 128, nsmax * 128), dtype=ml_dtypes.bfloat16)
    iota[0] = np.broadcast_to(
        np.tile(np.arange(128, dtype=np.float32), nsmax), (128, nsmax * 128))
    iota[1] = np.broadcast_to(
        np.tile(np.arange(128, 256, dtype=np.float32), nsmax),
        (128, nsmax * 128))

    common = {
        "WqT": np.ascontiguousarray(np.asarray(Wq, np.float32).T),
        "WkT": np.ascontiguousarray(np.asarray(Wk, np.float32).T),
        "WvT": np.ascontiguousarray(np.asarray(Wv, np.float32).T),
        "W1T": np.ascontiguousarray(np.asarray(W1, np.float32).T),
        "W2T": np.ascontiguousarray(np.asarray(W2, np.float32).T),
        "b1": np.asarray(b1, np.float32).reshape(128, 1),
        "b2": np.asarray(b2, np.float32).reshape(128, 1),
        "ident": np.eye(128, dtype=np.float32),
        "iota": iota,
    }
    in_maps = []
    for r in range(CORES):
        m = dict(common)
        m["featsT"] = np.ascontiguousarray(feats[r * NSH:(r + 1) * NSH].T)
        m["qv_idx"] = per_core[r]["qv_idx"]
        m["jif"] = per_core[r]["jif"]
        m["st"] = per_core[r]["st"]
        in_maps.append(m)

    res = run_bass_kernel_spmd(nc, in_maps, core_ids=list(range(CORES)))
    global _LAST_RESULTS
    _LAST_RESULTS = res
    out = np.concatenate(
        [np.asarray(res.results[r]["out"]) for r in range(CORES)], axis=0)
    return out.astype(np.float32)


_LAST_RESULTS = None
